# revision 1
# baseline (speedup 1.0000x reference)
"""Trainium2 Bass kernel for nn_CTransformer (2-layer coordinate-bias transformer).

Strategy (8 NeuronCores, SPMD):
- N=401 rows (CLS + 400 patches; 400 is a perfect square so no wrap pad),
  padded to 408 = 8*51. Core r owns query rows [51r, 51r+51).
- fc1 row-sharded; AllGather h1 -> every core has the full sequence.
- Layer-1 attention row-sharded: each core computes cpb bias / scores /
  softmax / AV / proj for its 51 query rows.  K/V (all rows) are computed
  replicated from the gathered h1 (cheaper than a second collective).
- AllGather h2.  Only the CLS row of layer 2's output feeds the final head,
  so layer 2 collapses to: K/V for all rows + a single query row (replicated).
- Final layernorm + 2-way head run on host (512 floats).
- LN gains are folded into qkv weights on host; attention scale folded into
  the q columns; cpb second-layer bias is softmax-invariant and dropped.
- Matmul operands are float32r (full-rate fp32 storage); accumulation fp32.
"""
import os

import numpy as np

import concourse.bass as bass
import concourse.mybir as mybir
import concourse.tile as tile
from concourse import bacc
from concourse.bass_utils import run_bass_kernel_spmd
from concourse.masks import make_identity

# ---- problem constants (hardcoded per contract) ----
DIM = 512
HEADS = 8
HD = 64
SCALE = HD ** -0.5
HIN = 1024
NREAL = 401
NPAD = 408
R = 51
NCORES = 8
CPB = 512
LN_EPS = 1e-5
NEG = -1e30
L = R * NPAD            # 20808
LB = 512
NLB = (L + LB - 1) // LB  # 41
LPAD = NLB * LB           # 20992

F32 = mybir.dt.float32
F32R = mybir.dt.float32r
AX = mybir.AxisListType
ALU = mybir.AluOpType
ACT = mybir.ActivationFunctionType

MMDT = F32R  # dtype for matmul operands


def _layernorm_tile(nc, pools, out, in_, pr, eps_sb):
    """LN over free axis (512) for in_[:pr] -> out[:pr] (out may be MMDT)."""
    small = pools["small"]
    stats = small.tile([128, 6], F32, name="stats", tag="stats")
    mv = small.tile([128, 2], F32, name="mv", tag="mv")
    nc.vector.bn_stats(out=stats[:pr], in_=in_[:pr])
    nc.vector.bn_aggr(out=mv[:pr], in_=stats[:pr])
    std = small.tile([128, 1], F32, name="std", tag="std")
    nc.scalar.activation(std[:pr], mv[:pr, 1:2], ACT.Sqrt, bias=eps_sb[:pr])
    istd = small.tile([128, 1], F32, name="istd", tag="istd")
    nc.vector.reciprocal(istd[:pr], std[:pr])
    nc.vector.tensor_scalar(
        out=out[:pr], in0=in_[:pr],
        scalar1=mv[:pr, 0:1], scalar2=istd[:pr],
        op0=ALU.subtract, op1=ALU.mult,
    )


def build(sim_local=False):
    nc = bacc.Bacc(None, target_bir_lowering=False, num_devices=NCORES)

    # ---- DRAM I/O ----
    # per-core
    hT_own = nc.dram_tensor("hT_own", [HIN, R], F32, kind="ExternalInput")
    c_own = nc.dram_tensor("c_own", [R, 2], F32, kind="ExternalInput")
    mask_own = nc.dram_tensor("mask_own", [R, 1], F32, kind="ExternalInput")
    clsadd = nc.dram_tensor("clsadd", [R, DIM], F32, kind="ExternalInput")
    # replicated
    coordsT = nc.dram_tensor("coordsT", [2, NPAD], F32, kind="ExternalInput")
    rel2nT = nc.dram_tensor("rel2nT", [2, NPAD], F32, kind="ExternalInput")
    fc1_w = nc.dram_tensor("fc1_w", [HIN, DIM], F32, kind="ExternalInput")
    fc1_b = nc.dram_tensor("fc1_b", [1, DIM], F32, kind="ExternalInput")
    qkv_w = [nc.dram_tensor(f"qkv_w{i}", [DIM, 3 * DIM], F32, kind="ExternalInput") for i in (1, 2)]
    bTqk = [nc.dram_tensor(f"bTqk{i}", [128, 8], F32, kind="ExternalInput") for i in (1, 2)]
    bv = [nc.dram_tensor(f"bv{i}", [1, DIM], F32, kind="ExternalInput") for i in (1, 2)]
    proj_w = [nc.dram_tensor(f"proj_w{i}", [DIM, DIM], F32, kind="ExternalInput") for i in (1, 2)]
    proj_b = [nc.dram_tensor(f"proj_b{i}", [1, DIM], F32, kind="ExternalInput") for i in (1, 2)]
    cpb_w1 = [nc.dram_tensor(f"cpb_w1{i}", [2, CPB], F32, kind="ExternalInput") for i in (1, 2)]
    cpb_b1T = [nc.dram_tensor(f"cpb_b1T{i}", [128, 4], F32, kind="ExternalInput") for i in (1, 2)]
    cpb_w2 = [nc.dram_tensor(f"cpb_w2{i}", [CPB, HEADS], F32, kind="ExternalInput") for i in (1, 2)]

    out_row = nc.dram_tensor("out_row", [1, DIM], F32, kind="ExternalOutput")

    ROWT = [128, 128, 128, 24]  # row-tile partition counts for 408 rows
    R2 = 52  # even-padded own-row count (f32r moving dim must be even)

    with tile.TileContext(nc) as tc:
        with (
            tc.tile_pool(name="wt", bufs=1) as wt,
            tc.tile_pool(name="actp", bufs=1) as ap_,
            tc.tile_pool(name="big1", bufs=1) as big1,
            tc.tile_pool(name="dbl", bufs=2) as dbl,
            tc.tile_pool(name="small", bufs=4) as small,
            tc.tile_pool(name="dram", bufs=1, space="DRAM") as dram,
        ):
            pools = {"small": small}

            # ================= weights / constants =================
            ident_f = wt.tile([128, 128], F32, name="ident_f", tag="identf")
            make_identity(nc, ident_f[:])
            ident = wt.tile([128, 128], MMDT, name="ident", tag="ident")
            nc.vector.tensor_copy(ident[:], ident_f[:])
            eps_sb = wt.tile([128, 1], F32, name="eps_sb", tag="eps")
            nc.vector.memset(eps_sb[:], LN_EPS)

            fc1w_sb = wt.tile([128, 8, DIM], MMDT, name="fc1w_sb", tag="fc1w")
            nc.sync.dma_start(fc1w_sb[:], fc1_w[:].bitcast(MMDT).rearrange("(o p) f -> p o f", p=128))
            hT_sb = ap_.tile([128, 8, R], MMDT, name="hT_sb", tag="hT")
            nc.sync.dma_start(hT_sb[:], hT_own[:].bitcast(MMDT).rearrange("(o p) f -> p o f", p=128))

            qkvw_sb = big1.tile([128, 4, 3 * DIM], MMDT, name="qkvw1_sb", tag="qkvw")
            nc.sync.dma_start(qkvw_sb[:], qkv_w[0][:].bitcast(MMDT).rearrange("(o p) f -> p o f", p=128))
            projw_sb = big1.tile([128, 4, DIM], MMDT, name="projw1_sb", tag="projw")
            nc.sync.dma_start(projw_sb[:], proj_w[0][:].bitcast(MMDT).rearrange("(o p) f -> p o f", p=128))

            cpbw1_sb = [wt.tile([2, CPB], MMDT, name=f"cpbw1_{i}", tag=f"cpbw1_{i}") for i in range(2)]
            cpbb1_sb = [wt.tile([128, 4], F32, name=f"cpbb1_{i}", tag=f"cpbb1_{i}") for i in range(2)]
            cpbw2_sb = [wt.tile([128, 4, HEADS], MMDT, name=f"cpbw2_{i}", tag=f"cpbw2_{i}") for i in range(2)]
            for i in range(2):
                nc.sync.dma_start(cpbw1_sb[i][:], cpb_w1[i][:].bitcast(MMDT))
                nc.sync.dma_start(cpbb1_sb[i][:], cpb_b1T[i][:])
                nc.sync.dma_start(cpbw2_sb[i][:], cpb_w2[i][:].bitcast(MMDT).rearrange("(o p) f -> p o f", p=128))

            bTqk_sb = [wt.tile([128, 8], F32, name=f"bTqk_{i}", tag=f"bTqk_{i}") for i in range(2)]
            for i in range(2):
                nc.sync.dma_start(bTqk_sb[i][:], bTqk[i][:])

            coordsT_sb = wt.tile([2, NPAD], F32, name="coordsT_sb", tag="coordsT")
            nc.sync.dma_start(coordsT_sb[:], coordsT[:])
            cown_sb = wt.tile([R, 2], F32, name="cown_sb", tag="cown")
            nc.sync.dma_start(cown_sb[:], c_own[:])
            mask_sb = wt.tile([R, 1], F32, name="mask_sb", tag="mask")
            nc.sync.dma_start(mask_sb[:], mask_own[:])
            cls_sb = wt.tile([R, DIM], F32, name="cls_sb", tag="cls")
            nc.sync.dma_start(cls_sb[:], clsadd[:])

            def bcast(dst_pool, name, src, p, f):
                t = dst_pool.tile([p, f], F32, name=name, tag=name)
                src_ap = bass.AP(tensor=src.tensor if isinstance(src, bass.AP) else src,
                                 offset=src.offset if isinstance(src, bass.AP) else 0,
                                 ap=[[0, p]] + [list(x) for x in (src.ap[1:] if isinstance(src, bass.AP) else [[1, f]])])
                nc.gpsimd.dma_start(t[:], src_ap)
                return t

            fc1b_rep = bcast(wt, "fc1b_rep", fc1_b[:], R, DIM)
            projb_rep = [bcast(wt, f"projb_rep{i}", proj_b[i][:], R if i == 0 else 1, DIM) for i in range(2)]
            vb_rep = [bcast(wt, f"vb_rep{i}", bv[i][:], 128, DIM) for i in range(2)]
            cxb = bcast(wt, "cxb", coordsT[0:1, :], R, NPAD)
            cyb = bcast(wt, "cyb", coordsT[1:2, :], R, NPAD)

            # ================= rel + flatten (layer-1, own rows) =================
            relx = ap_.tile([R, NPAD], F32, name="relx", tag="relx")
            rely = ap_.tile([R, NPAD], F32, name="rely", tag="rely")
            nc.vector.tensor_scalar(out=relx[:], in0=cxb[:], scalar1=cown_sb[:, 0:1],
                                    scalar2=-1.0, op0=ALU.subtract, op1=ALU.mult)
            nc.vector.tensor_scalar(out=rely[:], in0=cyb[:], scalar1=cown_sb[:, 1:2],
                                    scalar2=-1.0, op0=ALU.subtract, op1=ALU.mult)
            nsq = ap_.tile([R, NPAD], F32, name="nsq", tag="nsq")
            t2 = ap_.tile([R, NPAD], F32, name="t2", tag="t2")
            nc.vector.tensor_tensor(nsq[:], relx[:], relx[:], ALU.mult)
            nc.vector.tensor_tensor(t2[:], rely[:], rely[:], ALU.mult)
            nc.vector.tensor_tensor(nsq[:], nsq[:], t2[:], ALU.add)
            nc.scalar.activation(nsq[:], nsq[:], ACT.Sqrt)       # norm
            nc.vector.tensor_scalar_add(nsq[:], nsq[:], 1e-6)    # norm + 1e-6
            inv = ap_.tile([R, NPAD], F32, name="invn", tag="invn")
            nc.vector.reciprocal(inv[:], nsq[:])
            relxn = ap_.tile([R, NPAD], MMDT, name="relxn", tag="relxn")
            relyn = ap_.tile([R, NPAD], MMDT, name="relyn", tag="relyn")
            nc.vector.tensor_tensor(relxn[:], relx[:], inv[:], ALU.mult)
            nc.vector.tensor_tensor(relyn[:], rely[:], inv[:], ALU.mult)
            # flatten via DRAM bounce; cpb loop streams (2, 512) blocks back in
            reld = dram.tile([2, R, NPAD], F32, name="reld")
            nc.sync.dma_start(reld[0].bitcast(MMDT), relxn[:])
            nc.sync.dma_start(reld[1].bitcast(MMDT), relyn[:])
            reld_flat = reld[:].bitcast(MMDT).rearrange("c p f -> c (p f)")  # (2, L)

            # ================= fc1 (own rows) + AG1 =================
            with tc.tile_pool(name="ps0", bufs=2, space="PSUM") as ps0:
                fc1_ps = ps0.tile([R, DIM], F32, name="fc1_ps", tag="pfc")
                for k in range(8):
                    nc.tensor.matmul(fc1_ps[:], hT_sb[:, k, :], fc1w_sb[:, k, :],
                                     start=(k == 0), stop=(k == 7))
                h1_own = ap_.tile([R, DIM], F32, name="h1_own", tag="h1own")
                nc.vector.tensor_tensor(h1_own[:], fc1_ps[:], fc1b_rep[:], ALU.add)
                nc.scalar.activation(h1_own[:], h1_own[:], ACT.Relu, scale=mask_sb[:])
                nc.vector.tensor_tensor(h1_own[:], h1_own[:], cls_sb[:], ALU.add)

            ag1_in = dram.tile([R, DIM], F32, name="ag1_in")
            ag1_out = dram.tile([NPAD, DIM], F32, name="ag1_out", addr_space="Shared")
            nc.sync.dma_start(ag1_in[:], h1_own[:])
            if sim_local:
                nc.sync.dma_start(ag1_out[0:R, :], ag1_in[:])
            else:
                nc.gpsimd.collective_compute(
                    "AllGather", ALU.bypass, replica_groups=[list(range(NCORES))],
                    ins=[ag1_in[:].opt()], outs=[ag1_out[:].opt()],
                )

            def load_hf(ag_out, name):
                hf = big1.tile([128, 4, DIM], F32, name=name, tag="hf")
                nc.sync.dma_start(hf[:, 0:3, :], ag_out[0:384, :].rearrange("(o p) f -> p o f", p=128))
                nc.sync.dma_start(hf[0:24, 3, :], ag_out[384:NPAD, :])
                return hf

            h1f = load_hf(ag1_out, "h1f")

            # ============ LN + transpose + qkv (layer 1) ============
            def make_xT(hf, xt_name):
                """LN rows of hf then transpose -> xT (128, 4 dchunk, NPAD rows)."""
                xT = big1.tile([128, 4, NPAD], MMDT, name=xt_name, tag="xT")
                with tc.tile_pool(name="pstr", bufs=2, space="PSUM") as pstr:
                    for t in range(4):
                        pr = ROWT[t]
                        x_t = dbl.tile([128, DIM], MMDT, name=f"x_{xt_name}_{t}", tag="xrow")
                        _layernorm_tile(nc, pools, x_t, hf[:, t, :], pr, eps_sb)
                        for dc in range(4):
                            tp = pstr.tile([128, 128], MMDT, name=f"tp_{xt_name}_{t}_{dc}", tag="ptr")
                            nc.tensor.transpose(tp[:, :pr], x_t[:pr, dc * 128:(dc + 1) * 128], ident[:pr, :pr])
                            nc.vector.tensor_copy(xT[:, dc, t * 128:t * 128 + pr], tp[:, :pr])
                return xT

            xT = make_xT(h1f, "x1T")

            # own-row LN + transpose -> x_ownT (for q)
            x_own = ap_.tile([R, DIM], MMDT, name="x_own", tag="xown")
            _layernorm_tile(nc, pools, x_own, h1_own, R, eps_sb)
            x_ownT = ap_.tile([128, 4, R2], MMDT, name="x_ownT", tag="xownT")
            with tc.tile_pool(name="psq", bufs=2, space="PSUM") as psq:
                for dc in range(4):
                    tp = psq.tile([128, R2], MMDT, name=f"tpq_{dc}", tag="ptrq")
                    nc.tensor.transpose(tp[:, :R2], x_own[:R, dc * 128:(dc + 1) * 128], ident[:R, :R2])
                    nc.vector.tensor_copy(x_ownT[:, dc, :], tp[:, :R2])

            def make_qkv(xT_full, xT_q, qname, kname, vname, li):
                """qT (own rows), kT (all rows), v (all rows, natural)."""
                qT = ap_.tile([128, 4, R2], MMDT, name=qname, tag="qT")
                kT = big1.tile([128, 4, NPAD], MMDT, name=kname, tag="kT")
                v = big1.tile([128, 4, DIM], F32, name=vname, tag="v")
                with tc.tile_pool(name=f"psqkv{li}", bufs=2, space="PSUM") as psk:
                    for mc in range(4):
                        qps = psk.tile([128, R2], F32, name=f"qps{li}_{mc}", tag="qps")
                        for kc in range(4):
                            nc.tensor.matmul(qps[:], qkvw_sb[:, kc, mc * 128:(mc + 1) * 128],
                                             xT_q[:, kc, :], start=(kc == 0), stop=(kc == 3))
                        nc.scalar.activation(qT[:, mc, :], qps[:], ACT.Identity,
                                             bias=bTqk_sb[li][:, mc:mc + 1])
                    for mc in range(4):
                        kps = psk.tile([128, NPAD], F32, name=f"kps{li}_{mc}", tag="kps")
                        for kc in range(4):
                            nc.tensor.matmul(kps[:], qkvw_sb[:, kc, DIM + mc * 128:DIM + (mc + 1) * 128],
                                             xT_full[:, kc, :], start=(kc == 0), stop=(kc == 3))
                        nc.scalar.activation(kT[:, mc, :], kps[:], ACT.Identity,
                                             bias=bTqk_sb[li][:, 4 + mc:5 + mc])
                    for rt in range(4):
                        pr = ROWT[rt]
                        vps = psk.tile([128, DIM], F32, name=f"vps{li}_{rt}", tag="vps")
                        for kc in range(4):
                            nc.tensor.matmul(vps[:pr], xT_full[:, kc, rt * 128:rt * 128 + pr],
                                             qkvw_sb[:, kc, 2 * DIM:3 * DIM], start=(kc == 0), stop=(kc == 3))
                        nc.vector.tensor_tensor(v[:pr, rt, :], vps[:pr], vb_rep[li][:pr], ALU.add)
                return qT, kT, v

            qT, kT, v = make_qkv(xT, x_ownT, "q1T", "k1T", "v1", 0)

            # ================= cpb bias (layer 1, own rows) =================
            biasd = dram.tile([HEADS, L], F32, name="biasd")
            with (
                tc.tile_pool(name="psh", bufs=4, space="PSUM") as psh,
                tc.tile_pool(name="psb", bufs=3, space="PSUM") as psb,
                tc.tile_pool(name="relp", bufs=4) as relp,
            ):
                for lb in range(NLB):
                    n = min(LB, L - lb * LB)
                    relb = relp.tile([2, LB], MMDT, name=f"relb_{lb}", tag="relb")
                    if n < LB:
                        nc.vector.memset(relb[:].bitcast(F32), 0.0)
                    nc.sync.dma_start(relb[:, 0:n], reld_flat[:, lb * LB:lb * LB + n])
                    hs = []
                    for k in range(4):
                        hp = psh.tile([128, LB], F32, name=f"hp_{lb}_{k}", tag="hp")
                        nc.tensor.matmul(hp[:], cpbw1_sb[0][:, k * 128:(k + 1) * 128],
                                         relb[:], start=True, stop=True)
                        h_sb = dbl.tile([128, LB], MMDT, name=f"hs_{lb}_{k}", tag=f"hs{k}")
                        if k % 2 == 0:
                            nc.scalar.activation(h_sb[:], hp[:], ACT.Relu,
                                                 bias=cpbb1_sb[0][:, k:k + 1])
                        else:
                            nc.vector.tensor_scalar(out=h_sb[:], in0=hp[:],
                                                    scalar1=cpbb1_sb[0][:, k:k + 1], scalar2=0.0,
                                                    op0=ALU.add, op1=ALU.max)
                        hs.append(h_sb)
                    bp = psb.tile([HEADS, LB], F32, name=f"bp_{lb}", tag="bp")
                    for k in range(4):
                        nc.tensor.matmul(bp[:], cpbw2_sb[0][:, k, :], hs[k][:],
                                         start=(k == 0), stop=(k == 3))
                    bpsb = relp.tile([HEADS, LB], F32, name=f"bpsb_{lb}", tag="bpsb")
                    nc.scalar.copy(bpsb[:], bp[:])
                    nc.sync.dma_start(biasd[:, lb * LB:lb * LB + n], bpsb[:, 0:n])

            # unflatten from DRAM: (8, L) -> per-head (51, 408)
            bias_h = ap_.tile([R, HEADS, NPAD], F32, name="bias_h", tag="bias_h")
            for h in range(HEADS):
                nc.sync.dma_start(bias_h[:, h, :], biasd[h, :].rearrange("(p f) -> p f", f=NPAD))

            # ================= attention heads (layer 1, own rows) =================
            aoT = ap_.tile([128, 4, R2], MMDT, name="aoT", tag="aoT")
            with (
                tc.tile_pool(name="pssc", bufs=2, space="PSUM") as pssc,
                tc.tile_pool(name="pstr2", bufs=2, space="PSUM") as pstr2,
                tc.tile_pool(name="psav", bufs=1, space="PSUM") as psav,
            ):
                pav = psav.tile([128, 4, R2], F32, name="pav", tag="pav")
                for h in range(HEADS):
                    hc, hp_ = divmod(h, 2)
                    sp = pssc.tile([R2, NPAD], F32, name=f"sp_{h}", tag="sp")
                    nc.tensor.matmul(sp[:], qT[hp_ * 64:(hp_ + 1) * 64, hc, :],
                                     kT[hp_ * 64:(hp_ + 1) * 64, hc, :], start=True, stop=True)
                    s_sb = dbl.tile([R2, NPAD], F32, name=f"s_{h}", tag="s_sb")
                    nc.vector.tensor_copy(s_sb[:], sp[:])
                    nc.vector.tensor_tensor(s_sb[:R], s_sb[:R], bias_h[:, h, :], ALU.add)
                    nc.vector.memset(s_sb[:, NREAL:NPAD], NEG)
                    mx = small.tile([R2, 1], F32, name=f"mx_{h}", tag="mx")
                    nc.vector.tensor_reduce(mx[:], s_sb[:], axis=AX.X, op=ALU.max, negate=True)
                    p_sb = dbl.tile([R2, NPAD], F32, name=f"p_{h}", tag="p_sb")
                    ssum = small.tile([R2, 1], F32, name=f"ssum_{h}", tag="ssum")
                    nc.scalar.activation(p_sb[:], s_sb[:], ACT.Exp, bias=mx[:], accum_out=ssum[:])
                    sinv = small.tile([R2, 1], F32, name=f"sinv_{h}", tag="sinv")
                    nc.vector.reciprocal(sinv[:], ssum[:])
                    attn = dbl.tile([R2, NPAD], F32, name=f"attn_{h}", tag="attn")
                    nc.vector.tensor_scalar_mul(attn[:], p_sb[:], sinv[:])
                    for jc in range(4):
                        jK = ROWT[jc]
                        tp = pstr2.tile([128, R2], F32, name=f"ptp_{h}_{jc}", tag="ptp")
                        nc.tensor.transpose(tp[:jK, :], attn[:, jc * 128:jc * 128 + jK], ident_f[:R2, :R2])
                        pt_sb = dbl.tile([128, R2], F32, name=f"pt_{h}_{jc}", tag="pt_sb")
                        nc.vector.tensor_copy(pt_sb[:jK, :], tp[:jK, :])
                        nc.tensor.matmul(pav[hp_ * 64:(hp_ + 1) * 64, hc, :],
                                         v[0:jK, jc, h * 64:(h + 1) * 64], pt_sb[:jK, :],
                                         start=(jc == 0), stop=(jc == 3))
                for c in range(4):
                    nc.vector.tensor_copy(aoT[:, c, :], pav[:, c, :])

            # ================= proj + residual + AG2 =================
            with tc.tile_pool(name="pspr", bufs=2, space="PSUM") as pspr:
                pr_ps = pspr.tile([R2, DIM], F32, name="pr_ps", tag="prps")
                for c in range(4):
                    nc.tensor.matmul(pr_ps[:], aoT[:, c, :], projw_sb[:, c, :],
                                     start=(c == 0), stop=(c == 3))
                h2_own = ap_.tile([R, DIM], F32, name="h2_own", tag="h2own")
                nc.vector.tensor_tensor(h2_own[:], pr_ps[:R], projb_rep[0][:], ALU.add)
                nc.vector.tensor_tensor(h2_own[:], h2_own[:], h1_own[:], ALU.add)

            ag2_in = dram.tile([R, DIM], F32, name="ag2_in")
            ag2_out = dram.tile([NPAD, DIM], F32, name="ag2_out", addr_space="Shared")
            nc.sync.dma_start(ag2_in[:], h2_own[:])
            if sim_local:
                nc.sync.dma_start(ag2_out[0:R, :], ag2_in[:])
            else:
                nc.gpsimd.collective_compute(
                    "AllGather", ALU.bypass, replica_groups=[list(range(NCORES))],
                    ins=[ag2_in[:].opt()], outs=[ag2_out[:].opt()],
                )
            h2f = load_hf(ag2_out, "h2f")

            # ================= layer 2 (replicated; only CLS row matters) ====
            # reload weights for layer 2 into the same tags
            qkvw2_sb = big1.tile([128, 4, 3 * DIM], MMDT, name="qkvw2_sb", tag="qkvw")
            nc.sync.dma_start(qkvw2_sb[:], qkv_w[1][:].bitcast(MMDT).rearrange("(o p) f -> p o f", p=128))
            projw2_sb = big1.tile([128, 4, DIM], MMDT, name="projw2_sb", tag="projw")
            nc.sync.dma_start(projw2_sb[:], proj_w[1][:].bitcast(MMDT).rearrange("(o p) f -> p o f", p=128))

            x2T = make_xT(h2f, "x2T")

            kT2 = big1.tile([128, 4, NPAD], MMDT, name="k2T", tag="kT")
            v2 = big1.tile([128, 4, DIM], F32, name="v2", tag="v")
            q2T = ap_.tile([128, 4, 1], F32, name="q2T", tag="q2T")
            with tc.tile_pool(name="psl2", bufs=2, space="PSUM") as psk:
                for mc in range(4):
                    kps = psk.tile([128, NPAD], F32, name=f"k2ps_{mc}", tag="k2ps")
                    for kc in range(4):
                        nc.tensor.matmul(kps[:], qkvw2_sb[:, kc, DIM + mc * 128:DIM + (mc + 1) * 128],
                                         x2T[:, kc, :], start=(kc == 0), stop=(kc == 3))
                    nc.scalar.activation(kT2[:, mc, :], kps[:], ACT.Identity,
                                         bias=bTqk_sb[1][:, 4 + mc:5 + mc])
                for rt in range(4):
                    pr = ROWT[rt]
                    vps = psk.tile([128, DIM], F32, name=f"v2ps_{rt}", tag="v2ps")
                    for kc in range(4):
                        nc.tensor.matmul(vps[:pr], x2T[:, kc, rt * 128:rt * 128 + pr],
                                         qkvw2_sb[:, kc, 2 * DIM:3 * DIM], start=(kc == 0), stop=(kc == 3))
                    nc.vector.tensor_tensor(v2[:pr, rt, :], vps[:pr], vb_rep[1][:pr], ALU.add)
                # q row 0 (CLS): rhs = x2T[:, :, 0:1]
                for mc in range(4):
                    q2ps = psk.tile([128, 2], F32, name=f"q2ps_{mc}", tag="q2ps")
                    for kc in range(4):
                        nc.tensor.matmul(q2ps[:], qkvw2_sb[:, kc, mc * 128:(mc + 1) * 128],
                                         x2T[:, kc, 0:2], start=(kc == 0), stop=(kc == 3))
                    nc.scalar.activation(q2T[:, mc, :], q2ps[:, 0:1], ACT.Identity,
                                         bias=bTqk_sb[1][:, mc:mc + 1])

            # block-diagonal q2 for fused scores2: (128, 4 chunks, 8 heads)
            q2bd = ap_.tile([128, 4, HEADS], MMDT, name="q2bd", tag="q2bd")
            nc.vector.memset(q2bd[:].bitcast(F32), 0.0)
            for h in range(HEADS):
                hc, hp_ = divmod(h, 2)
                nc.vector.tensor_copy(q2bd[hp_ * 64:(hp_ + 1) * 64, hc, h:h + 1],
                                      q2T[hp_ * 64:(hp_ + 1) * 64, hc, :])

            # rel2 row 0 (normalized (0,0)-c_j): host-precomputed input
            rel2n = ap_.tile([2, NPAD], MMDT, name="rel2n", tag="rel2n")
            nc.sync.dma_start(rel2n[:], rel2nT[:].bitcast(MMDT))

            with (
                tc.tile_pool(name="psh2", bufs=2, space="PSUM") as psh2,
                tc.tile_pool(name="pss2", bufs=1, space="PSUM") as pss2,
                tc.tile_pool(name="pst2", bufs=2, space="PSUM") as pst2,
                tc.tile_pool(name="psa2", bufs=1, space="PSUM") as psa2,
            ):
                hs2 = ap_.tile([128, 4, NPAD], MMDT, name="hs2", tag="hs2")
                for k in range(4):
                    hp2 = psh2.tile([128, NPAD], F32, name=f"hp2_{k}", tag="hp2")
                    nc.tensor.matmul(hp2[:], cpbw1_sb[1][:, k * 128:(k + 1) * 128],
                                     rel2n[:], start=True, stop=True)
                    nc.scalar.activation(hs2[:, k, :], hp2[:], ACT.Relu,
                                         bias=cpbb1_sb[1][:, k:k + 1])
                s2p = pss2.tile([HEADS, NPAD], F32, name="s2p", tag="s2p")
                for kc in range(4):
                    nc.tensor.matmul(s2p[:], q2bd[:, kc, :], kT2[:, kc, :],
                                     start=(kc == 0), stop=False)
                for kc in range(4):
                    nc.tensor.matmul(s2p[:], cpbw2_sb[1][:, kc, :], hs2[:, kc, :],
                                     start=False, stop=(kc == 3))
                s2 = ap_.tile([HEADS, NPAD], F32, name="s2", tag="s2")
                nc.vector.tensor_copy(s2[:], s2p[:])
                nc.vector.memset(s2[:, NREAL:NPAD], NEG)
                mx2 = small.tile([HEADS, 1], F32, name="mx2", tag="mx2")
                nc.vector.tensor_reduce(mx2[:], s2[:], axis=AX.X, op=ALU.max, negate=True)
                p2 = ap_.tile([HEADS, NPAD], F32, name="p2", tag="p2")
                ssum2 = small.tile([HEADS, 1], F32, name="ssum2", tag="ssum2")
                nc.scalar.activation(p2[:], s2[:], ACT.Exp, bias=mx2[:], accum_out=ssum2[:])
                sinv2 = small.tile([HEADS, 1], F32, name="sinv2", tag="sinv2")
                nc.vector.reciprocal(sinv2[:], ssum2[:])
                attn2 = ap_.tile([HEADS, NPAD], F32, name="attn2", tag="attn2")
                nc.vector.tensor_scalar_mul(attn2[:], p2[:], sinv2[:])

                PT2 = ap_.tile([128, 4, 10], F32, name="PT2", tag="PT2")
                nc.vector.memset(PT2[:], 0.0)
                for jc in range(4):
                    jK = ROWT[jc]
                    tp2 = pst2.tile([128, HEADS], F32, name=f"tp2_{jc}", tag="tp2")
                    nc.tensor.transpose(tp2[:jK, :], attn2[:, jc * 128:jc * 128 + jK],
                                        ident_f[:HEADS, :HEADS])
                    nc.vector.tensor_copy(PT2[:jK, jc, 0:HEADS], tp2[:jK, :])
                pav2 = psa2.tile([128, 8], F32, name="pav2", tag="pav2")
                for h in range(HEADS):
                    hc, hp_ = divmod(h, 2)
                    for jc in range(4):
                        jK = ROWT[jc]
                        nc.tensor.matmul(pav2[hp_ * 64:(hp_ + 1) * 64, 2 * hc:2 * hc + 2],
                                         v2[0:jK, jc, h * 64:(h + 1) * 64],
                                         PT2[0:jK, jc, h:h + 2],
                                         start=(jc == 0), stop=(jc == 3))
                o2T = ap_.tile([128, 4, 1], MMDT, name="o2T", tag="o2T")
                for c in range(4):
                    nc.vector.tensor_copy(o2T[:, c, :], pav2[:, 2 * c:2 * c + 1])

            with tc.tile_pool(name="psp2", bufs=1, space="PSUM") as psp2:
                pr2 = psp2.tile([1, DIM], F32, name="pr2", tag="pr2")
                for c in range(4):
                    nc.tensor.matmul(pr2[:], o2T[:, c, :], projw2_sb[:, c, :],
                                     start=(c == 0), stop=(c == 3))
                orow = ap_.tile([1, DIM], F32, name="orow", tag="orow")
                nc.vector.tensor_tensor(orow[0:1, :], pr2[0:1, :], projb_rep[1][0:1, :], ALU.add)
                nc.vector.tensor_tensor(orow[0:1, :], orow[0:1, :], h2f[0:1, 0, :], ALU.add)
                nc.sync.dma_start(out_row[:], orow[0:1, :])

    nc.finalize()
    return nc


_NC_CACHE = None


def _get_nc():
    global _NC_CACHE
    if _NC_CACHE is None:
        _NC_CACHE = build()
    return _NC_CACHE


def _prep_inputs(inputs):
    """Host-side sharding + weight folding. Returns per-core in_maps."""
    f = np.float32
    h = np.asarray(inputs["h"][0], f)              # (400, 1024)
    coords = np.asarray(inputs["coords"][0], f)    # (400, 2)

    coords_full = np.zeros((NPAD, 2), f)
    coords_full[1:1 + 400] = coords

    hT_full = np.zeros((HIN, NPAD), f)
    hT_full[:, 1:1 + 400] = h.T

    def fold(p):
        g = np.asarray(inputs[p + "_norm_g"], f)
        b = np.asarray(inputs[p + "_norm_b"], f)
        qw = np.asarray(inputs[p + "_qkv_w"], f)
        qb = np.asarray(inputs[p + "_qkv_b"], f)
        qkv_w = (g[:, None] * qw).astype(f)
        qkv_b = (b @ qw + qb).astype(f)
        qkv_w[:, :DIM] *= SCALE
        qkv_b[:DIM] *= SCALE
        return qkv_w, qkv_b

    qkv_w1, qkv_b1 = fold("l1")
    qkv_w2, qkv_b2 = fold("l2")

    nrm = np.sqrt((coords_full ** 2).sum(1))
    rel2n_host = (-coords_full / (nrm + 1e-6)[:, None]).astype(f)

    rep = {
        "coordsT": np.ascontiguousarray(coords_full.T),
        "rel2nT": np.ascontiguousarray(rel2n_host.T),
        "fc1_w": np.asarray(inputs["fc1_w"], f),
        "fc1_b": np.asarray(inputs["fc1_b"], f)[None, :],
        "qkv_w1": qkv_w1, "qkv_w2": qkv_w2,
        "bTqk1": np.ascontiguousarray(qkv_b1[:2 * DIM].reshape(8, 128).T),
        "bTqk2": np.ascontiguousarray(qkv_b2[:2 * DIM].reshape(8, 128).T),
        "bv1": qkv_b1[2 * DIM:][None, :], "bv2": qkv_b2[2 * DIM:][None, :],
        "proj_w1": np.asarray(inputs["l1_proj_w"], f),
        "proj_w2": np.asarray(inputs["l2_proj_w"], f),
        "proj_b1": np.asarray(inputs["l1_proj_b"], f)[None, :],
        "proj_b2": np.asarray(inputs["l2_proj_b"], f)[None, :],
        "cpb_w11": np.asarray(inputs["l1_cpb_w1"], f),
        "cpb_w12": np.asarray(inputs["l2_cpb_w1"], f),
        "cpb_b1T1": np.ascontiguousarray(np.asarray(inputs["l1_cpb_b1"], f).reshape(4, 128).T),
        "cpb_b1T2": np.ascontiguousarray(np.asarray(inputs["l2_cpb_b1"], f).reshape(4, 128).T),
        "cpb_w21": np.asarray(inputs["l1_cpb_w2"], f),
        "cpb_w22": np.asarray(inputs["l2_cpb_w2"], f),
    }

    cls_token = np.asarray(inputs["cls_token"][0, 0], f)

    in_maps = []
    for r in range(NCORES):
        rows = slice(51 * r, 51 * r + 51)
        gidx = np.arange(51 * r, 51 * r + 51)
        mask = ((gidx >= 1) & (gidx <= 400)).astype(f)[:, None]
        cadd = np.zeros((R, DIM), f)
        if r == 0:
            cadd[0] = cls_token
        in_maps.append({
            "hT_own": np.ascontiguousarray(hT_full[:, rows]),
            "c_own": np.ascontiguousarray(coords_full[rows]),
            "mask_own": mask,
            "clsadd": cadd,
            "coordsT": rep["coordsT"], "rel2nT": rep["rel2nT"],
            "fc1_w": rep["fc1_w"], "fc1_b": rep["fc1_b"],
            "qkv_w1": rep["qkv_w1"], "qkv_w2": rep["qkv_w2"],
            "bTqk1": rep["bTqk1"], "bTqk2": rep["bTqk2"],
            "bv1": rep["bv1"], "bv2": rep["bv2"],
            "proj_w1": rep["proj_w1"], "proj_w2": rep["proj_w2"],
            "proj_b1": rep["proj_b1"], "proj_b2": rep["proj_b2"],
            "cpb_w11": rep["cpb_w11"], "cpb_w12": rep["cpb_w12"],
            "cpb_b1T1": rep["cpb_b1T1"], "cpb_b1T2": rep["cpb_b1T2"],
            "cpb_w21": rep["cpb_w21"], "cpb_w22": rep["cpb_w22"],
        })
    return in_maps


def run(inputs, **spmd_kwargs):
    nc = _get_nc()
    in_maps = _prep_inputs(inputs)
    res = run_bass_kernel_spmd(nc, in_maps, core_ids=list(range(NCORES)), **spmd_kwargs)
    row = res.results[0]["out_row"][0].astype(np.float64)  # (512,)

    # host final head: layernorm + fc2 on the CLS row
    g = np.asarray(inputs["norm_g"], np.float64)
    b = np.asarray(inputs["norm_b"], np.float64)
    m = row.mean()
    v = row.var()
    z = (row - m) / np.sqrt(v + LN_EPS) * g + b
    out = z @ np.asarray(inputs["fc2_w"], np.float64) + np.asarray(inputs["fc2_b"], np.float64)
    return out[None, :].astype(np.float32), res


def kernel(**inputs) -> np.ndarray:
    out, _ = run(inputs)
    return out

